# revision 29
# baseline (speedup 1.0000x reference)
"""Causal self-attention (q/k-swapped variant) Bass kernel for Trainium2.

Problem: B=2, T=2048, C=768, H=12, hs=64.
    k = x@Wk+bk ; q = x@Wq+bq ; v = x@Wv+bv          (per-head split)
    att[b,h,i,j] = (k[b,i,h,:] . q[b,j,h,:]) / 8     (note: k rows, q cols)
    att = softmax(causal-mask(att), axis=j)
    y = (att @ v) @ Wo + bo

Sharding: 8 cores = 2 batches x 4 head-groups (3 heads each).
Each core computes its 3 heads fully (QKV proj -> attention -> partial
output projection); host sums the 4 partial outputs per batch and adds bo.

All on-device score math is done in "transposed score" space: score tiles
have j (softmax axis) on partitions and i on the free dim, so the PV matmul
needs no transposes at all, and the softmax denominator falls out of the PV
matmul via an appended ones-column on V.
"""

import os
import sys

sys.path.insert(0, "/opt/trn_rl_repo")

import numpy as np

T = 2048
C = 768
HS = 64
HPC = 3          # heads per core
NCH = C // 128   # 6 contraction chunks
TB = T // 128    # 16 row blocks
JB = T // 128    # 16 j blocks
NCORES = 8
MM_DTYPE = os.environ.get("KERNEL_MM_DTYPE", "fp16")  # fp16 | bf16 | fp32

_cache = {}


def _segments(lo, hi):
    """Split [lo, hi) at 512 boundaries (PSUM bank / fp32 matmul N limit)."""
    out = []
    s = lo
    while s < hi:
        e = min((s // 512 + 1) * 512, hi)
        out.append((s, e))
        s = e
    return out


def _emit(ctx, tc):
    import concourse.bass as bass
    import concourse.tile as tile  # noqa: F401
    from concourse import mybir
    from concourse.bass import ts
    from concourse.masks import make_upper_triangular

    f32 = mybir.dt.float32
    mmd = {"fp16": mybir.dt.float16, "bf16": mybir.dt.bfloat16,
           "fp32": f32}[MM_DTYPE]  # matmul-input dtype
    nc = tc.nc

    xT = nc.dram_tensor("xT", (C, T), mmd, kind="ExternalInput").ap()
    wqk = nc.dram_tensor("wqk", (128, 3 * NCH * 128), mmd, kind="ExternalInput").ap()
    wv = nc.dram_tensor("wv", (128, NCH * 192), mmd, kind="ExternalInput").ap()
    wo01 = nc.dram_tensor("wo01", (128, C), mmd, kind="ExternalInput").ap()
    wo2 = nc.dram_tensor("wo2", (64, C), mmd, kind="ExternalInput").ap()
    bqk = nc.dram_tensor("bqk", (128, 3), f32, kind="ExternalInput").ap()
    bv = nc.dram_tensor("bv", (1, 192), f32, kind="ExternalInput").ap()
    y = nc.dram_tensor("y", (C, T), f32, kind="ExternalOutput").ap()  # transposed

    consts = ctx.enter_context(tc.tile_pool(name="consts", bufs=1))

    # ---- load inputs (wqk + xT first: they gate the first projections) ----
    wqk_sb = consts.tile([128, 3, NCH, 128], mmd)
    nc.sync.dma_start(wqk_sb[:], wqk.rearrange("p (g k m) -> p g k m", g=3, k=NCH))
    xT_sb = consts.tile([128, NCH, T], mmd)
    for k in range(NCH):
        for q in range(4):  # 4 column slices per chunk: faster first-arrival
            nc.sync.dma_start(xT_sb[:, k, ts(q, 512)],
                              xT[k * 128:(k + 1) * 128, ts(q, 512)])
    wv_sb = consts.tile([128, NCH, 192], mmd)
    nc.sync.dma_start(wv_sb[:], wv.rearrange("p (k m) -> p k m", k=NCH))
    wo01_sb = consts.tile([128, C], mmd)
    nc.sync.dma_start(wo01_sb[:], wo01)
    wo2_sb = consts.tile([64, C], mmd)
    nc.sync.dma_start(wo2_sb[:], wo2)
    bqk_sb = consts.tile([128, 3], f32)       # per-partition bias per QK group
    nc.sync.dma_start(bqk_sb[:], bqk)
    bvb_sb = consts.tile([128, 192], f32)     # bv broadcast across partitions
    nc.sync.dma_start(bvb_sb[:], bv.to_broadcast((128, 192)))

    scratch = consts.tile([128, 512], mmd)
    nc.vector.memset(scratch[:], 0.0)
    ones64 = consts.tile([1, 64], f32)
    nc.vector.memset(ones64[:], 1.0)
    trimask = consts.tile([128, 128], mmd)
    make_upper_triangular(nc, trimask[:], val=1.0, diag=True)

    V_aug = consts.tile([128, TB, HPC * 65], mmd)
    for h in range(HPC):
        nc.vector.memset(V_aug[:, :, h * 65 + 64:h * 65 + 65], 1.0)

    QK_sb = consts.tile([128, 3, T], mmd)     # g0=Q(h0,h1) g1=K(h0,h1) g2=[Q(h2)|K(h2)]
    KT2_sb = consts.tile([64, T], mmd)        # K(h2) shifted to base partition 0
    AT01_sb = consts.tile([128, T], mmd)      # normalized attn out, h0 rows 0:64, h1 64:128
    AT2_sb = consts.tile([64, T], mmd)        # normalized attn out, h2

    # ---- single fused pipeline ----
    # PSUM: psP (proj/outproj, 2 banks) + psS (scores, 4) + psO (Onum, 2) = 8
    psP = ctx.enter_context(tc.tile_pool(name="psP", bufs=2, space="PSUM"))
    psS = ctx.enter_context(tc.tile_pool(name="psS", bufs=1, space="PSUM"))
    psO = ctx.enter_context(tc.tile_pool(name="psO", bufs=1, space="PSUM"))
    sbE = ctx.enter_context(tc.tile_pool(name="E", bufs=3))
    sbATn = ctx.enter_context(tc.tile_pool(name="ATn", bufs=2))
    sbRZ = ctx.enter_context(tc.tile_pool(name="RZ", bufs=2))
    sbY = ctx.enter_context(tc.tile_pool(name="Y", bufs=4))

    # PE warm-up (keeps HAM at full clock while inputs stream in) + exp
    # table pre-load
    for _ in range(28):
        warm = psP.tile([128, 512], f32, tag="p")
        nc.tensor.matmul(warm[:], lhsT=scratch[:, 0:128], rhs=scratch[:],
                         start=True, stop=True, skip_group_check=True)
    edum = sbE.tile([128, 1024], mmd, tag="edum")
    nc.scalar.activation(edum[:, 0:512], scratch[:],
                         mybir.ActivationFunctionType.Exp, scale=0.125)

    def qk_mm(g, it):
        ps = psP.tile([128, 512], f32, tag="p")
        for k in range(NCH):
            nc.tensor.matmul(ps[:], lhsT=wqk_sb[:, g, k, :],
                             rhs=xT_sb[:, k, ts(it, 512)],
                             start=(k == 0), stop=(k == NCH - 1))
        return ps

    def qk_epi(ps, g, it):
        nc.vector.tensor_add(QK_sb[:, g, ts(it, 512)], ps[:],
                             bqk_sb[:, g:g + 1].to_broadcast((128, 512)))

    def qk_group(g, it):
        qk_epi(qk_mm(g, it), g, it)

    def v_mm(tb):
        ps = psP.tile([128, 512], f32, tag="p")
        for k in range(NCH):
            nc.tensor.matmul(ps[:, 0:192], lhsT=xT_sb[:, k, ts(tb, 128)],
                             rhs=wv_sb[:, k, :],
                             start=(k == 0), stop=(k == NCH - 1))
        return ps

    def v_epi(ps, tb):
        for h in range(HPC):
            nc.vector.tensor_add(V_aug[:, tb, h * 65:h * 65 + 64],
                                 ps[:, h * 64:(h + 1) * 64],
                                 bvb_sb[:, h * 64:(h + 1) * 64])

    def v_group(tb):
        v_epi(v_mm(tb), tb)

    def op_mm(cb, tt):
        ps = psP.tile([128, 512], f32, tag="p")
        nc.tensor.matmul(ps[:], lhsT=wo01_sb[:, ts(cb, 128)],
                         rhs=AT01_sb[:, ts(tt, 512)], start=True, stop=False)
        nc.tensor.matmul(ps[:], lhsT=wo2_sb[:, ts(cb, 128)],
                         rhs=AT2_sb[:, ts(tt, 512)], start=False, stop=True)
        return ps

    def op_epi(ps, cb, tt):
        ysb = sbY.tile([128, 512], f32)
        nc.vector.tensor_copy(ysb[:], ps[:])
        nc.sync.dma_start(y[cb * 128:(cb + 1) * 128, tt * 512:(tt + 1) * 512],
                          ysb[:])

    def kt2_shift(lo, hi):
        nc.sync.dma_start(KT2_sb[:, lo:hi], QK_sb[64:128, 2, lo:hi])

    # pre-phase: K^T cols 0:1023 of h0/h1, Q^T cols for jb 0-3, V blocks
    # 0-5 (unit h0's trailing PVs consume V(0..7) by its last chunk, and
    # deferred-epilogue fillers deliver at most v6/v7 in time); everything
    # else weaves into the chunk stream as PE filler.
    for it in range(2):
        qk_group(1, it)
    qk_group(0, 0)
    for tb in range(6):
        v_group(tb)

    from collections import deque
    # Filler groups are (mm_emitter, epi_maker) pairs. The mm stage runs PE
    # matmuls; the epilogue (a DVE bias-add / PSUM->SBUF copy) is deferred
    # two pops so it never enqueues on DVE while its matmuls are still in
    # flight (head-of-line blocking on the strict-FIFO DVE queue).
    # Emission-order invariants (Tile deps follow emission order):
    #  - qk(2,*)+kt2_shift before the half0-h2 unit (its STs read g2/KT2);
    #    they sit at pops 10-14, i.e. during half0-h1 -> barrier is a no-op.
    #  - qk(1,2..3) epi before half1's first ST (chunk 24): pops 3/5 +2.
    #  - v(tb) epi before the first PV reading V_aug[:,tb]: v8-15 pop at
    #    chunks 15-22, first consumer is half1-h1's PV(8) around chunk 35.
    def G(mm, epi=None):
        return (mm, epi)

    def qk_G(g, it):
        return G(lambda: qk_mm(g, it), lambda ps: qk_epi(ps, g, it))

    def v_G(tb):
        return G(lambda: v_mm(tb), lambda ps: v_epi(ps, tb))

    # kt2 shifts sit >=2 items after the qk(2,*) epis they read, so the
    # due-discipline of the epi backlog emits those adds first
    pre_fillers = deque(
        [qk_G(0, 1), v_G(6), v_G(7), qk_G(1, 2), qk_G(1, 3),
         qk_G(0, 2), qk_G(0, 3), qk_G(2, 0), qk_G(2, 1),
         qk_G(2, 2), qk_G(2, 3),
         G(lambda: (kt2_shift(0, 1024), None)[1]),
         G(lambda: (kt2_shift(1024, 2048), None)[1])]
    )
    v_late = deque([v_G(tb) for tb in range(8, TB)])
    op_fillers = deque()
    epi_backlog = deque()   # (due_popcount, thunk)
    popcnt = [0]

    def pop_filler():
        popcnt[0] += 1
        if epi_backlog and epi_backlog[0][0] <= popcnt[0]:
            epi_backlog.popleft()[1]()
            return True
        for q in (pre_fillers, v_late, op_fillers):
            if q:
                mm, epi = q.popleft()
                ps = mm()
                if epi is not None:
                    epi_backlog.append((popcnt[0] + 2, lambda: epi(ps)))
                return True
        if epi_backlog:
            epi_backlog.popleft()[1]()
            return True
        return False

    def flush_fillers(queues):
        for q in queues:
            while q:
                pop_filler()
        while epi_backlog:
            epi_backlog.popleft()[1]()

    # per-head (lhsT=Q^T, rhs=K^T) access patterns; partition bases match
    heads = [
        (QK_sb[0:64, 0, :], QK_sb[0:64, 1, :]),
        (QK_sb[64:128, 0, :], QK_sb[64:128, 1, :]),
        (QK_sb[0:64, 2, :], KT2_sb[:, :]),
    ]

    HW = 1024  # i-window per (half, head) unit
    # half1 runs h1 first so its AT-shift DMA clears long before the tail,
    # and ends on h0 whose normalization chain has no trailing shift.
    head_order = [(0, 1, 2), (1, 2, 0)]
    for half in range(T // HW):
        c0 = HW * half
        njb = (c0 + HW) // 128
        for h in head_order[half]:
            if h == 2 and (pre_fillers or epi_backlog):
                # h2 reads g2/KT2: force their writers (and any pending
                # epilogues) out now; normally a no-op by pop scheduling
                flush_fillers((pre_fillers,))
            QT, KT = heads[h]
            Onum = psO.tile([65, HW], f32)

            def emit_pv(jb, E2, par, lo):
                for a, b in _segments(lo, c0 + HW):
                    nc.tensor.matmul(Onum[:, a - c0:b - c0],
                                     lhsT=V_aug[:, jb, h * 65:(h + 1) * 65],
                                     rhs=E2[:, par, a - c0:b - c0],
                                     start=(jb == 0),
                                     stop=(jb == min(4 * (a // 512) + 3,
                                                     njb - 1)),
                                     skip_group_check=True)

            # chunks run in pairs sharing one 4-bank S tile and a single EXP
            # instruction (one 352-cycle ACT ramp per pair instead of two);
            # the odd chunk's 128 below-diagonal garbage columns are exp'd
            # too but never read by any PV.
            pending = []
            for jp in range(njb // 2):
                S2 = psS.tile([128, 2, HW], f32)
                E2 = sbE.tile([128, 2, HW], mmd)
                los = []
                for par in range(2):
                    jb = 2 * jp + par
                    lo = max(c0, 128 * jb)
                    los.append(lo)
                    for a, b in _segments(lo, c0 + HW):
                        nc.tensor.matmul(S2[:, par, a - c0:b - c0],
                                         lhsT=QT[:, ts(jb, 128)],
                                         rhs=KT[:, a:b], start=True, stop=True)
                    if not pop_filler():
                        # one dummy matmul per filler miss keeps the PE
                        # activity monitor from re-throttling mid-attention
                        warm = psP.tile([128, 512], f32, tag="p")
                        nc.tensor.matmul(warm[:], lhsT=scratch[:, 0:128],
                                         rhs=scratch[:], start=True, stop=True,
                                         skip_group_check=True)
                off = los[0] - c0
                nc.scalar.activation(E2[:, :, off:], S2[:, :, off:],
                                     mybir.ActivationFunctionType.Exp,
                                     scale=0.125)
                for par in range(2):
                    jb = 2 * jp + par
                    i0 = 128 * jb
                    if los[par] == i0:  # pair containing the diagonal block
                        r = i0 - c0
                        nc.vector.tensor_mul(E2[:, par, r:r + 128],
                                             E2[:, par, r:r + 128],
                                             trimask[:])
                    pending.append((jb, E2, par, los[par]))
                    while len(pending) > 3:  # PV trails ST by 3 chunks
                        emit_pv(*pending.pop(0))
            for item in pending:
                emit_pv(*item)

            # row 64 of Onum is Z, on one partition. DMA-reshape it straight
            # from PSUM to [128, HW/128] (overlapping the ATn copy) for a
            # parallel DVE reciprocal, fold back, then gpsimd replicates 1/Z
            # across 64 partitions for the divide.
            ATn = sbATn.tile([65, HW], f32)
            nc.vector.tensor_copy(ATn[:], Onum[:])
            z16 = sbRZ.tile([128, HW // 128], f32, tag="z16")
            nc.sync.dma_start(z16[:], ATn[64:65, :])
            r16 = sbRZ.tile([128, HW // 128], f32, tag="r16")
            nc.vector.reciprocal(r16[:], z16[:])
            rz1 = sbRZ.tile([1, HW], f32, tag="rz1")
            nc.sync.dma_start(rz1[:], r16[:])
            rzb = sbRZ.tile([64, HW], f32, tag="rzb")
            nc.gpsimd.partition_broadcast(rzb[:], rz1[:], channels=64)
            if h == 0:
                nc.vector.tensor_mul(AT01_sb[0:64, c0:c0 + HW], ATn[0:64, :],
                                     rzb[:])
            elif h == 2:
                nc.vector.tensor_mul(AT2_sb[:, c0:c0 + HW], ATn[0:64, :],
                                     rzb[:])
            else:
                # h1's rows live at partitions 64:128 of AT01: normalize into
                # a scratch tile, then partition-shift via SBUF-to-SBUF DMA.
                ATsh = sbRZ.tile([64, HW], mmd, tag="atsh")
                nc.vector.tensor_mul(ATsh[:], ATn[0:64, :], rzb[:])
                nc.sync.dma_start(AT01_sb[64:128, c0:c0 + HW], ATsh[:])

        # all heads done for this half: its output columns can project out;
        # groups run as fillers inside the next half (or drain at the end)
        for cb in range(NCH):
            for tt in range(c0 // 512, (c0 + HW) // 512):
                op_fillers.append(
                    G(lambda cb=cb, tt=tt: op_mm(cb, tt),
                      lambda ps, cb=cb, tt=tt: op_epi(ps, cb, tt)))

    # drain remaining fillers (the last half's output projection); a few
    # dummies bridge the last normalization chain so the PE stays warm
    for _ in range(8):
        warm = psP.tile([128, 512], f32, tag="p")
        nc.tensor.matmul(warm[:], lhsT=scratch[:, 0:128], rhs=scratch[:],
                         start=True, stop=True, skip_group_check=True)
    flush_fillers((pre_fillers, v_late, op_fillers))


def _build():
    if "nc" in _cache:
        return _cache["nc"]
    from contextlib import ExitStack

    import concourse.tile as tile
    from concourse import bacc

    nc = bacc.Bacc("TRN2", target_bir_lowering=False, debug=False,
                   num_devices=NCORES)
    with tile.TileContext(nc) as tc:
        with ExitStack() as ctx:
            _emit(ctx, tc)
    nc.compile()
    _cache["nc"] = nc
    return nc


def _install_trace_hooks():
    """Make trace=True work in this container: shim the missing
    antenv.axon_hooks NTFF-profile hook (ctypes into libaxon_pjrt.so) and
    skip the S3 artifact upload."""
    import contextlib
    import ctypes
    import types

    import concourse.bass_utils as bu

    bu.upload_artifacts = lambda tmpdir: tmpdir
    try:
        from antenv.axon_hooks import get_axon_ntff_profile_hook  # noqa: F401
        return
    except ImportError:
        pass

    so_path = "/opt/axon/libaxon_pjrt.so"
    if not os.path.exists(so_path):
        return
    lib = ctypes.CDLL(so_path)
    if not hasattr(lib, "axon_start_nrt_profile"):
        return
    lib.axon_start_nrt_profile.argtypes = [
        ctypes.POINTER(ctypes.c_int64), ctypes.c_size_t,
    ]
    lib.axon_start_nrt_profile.restype = ctypes.c_int64
    lib.axon_stop_nrt_profile.argtypes = [ctypes.c_char_p]
    lib.axon_stop_nrt_profile.restype = ctypes.c_int64

    @contextlib.contextmanager
    def _hook(output_dir, device_ids):
        import jax
        jax.devices()
        if device_ids:
            ids = (ctypes.c_int64 * len(device_ids))(*device_ids)
            rc = lib.axon_start_nrt_profile(ids, len(device_ids))
        else:
            rc = lib.axon_start_nrt_profile(None, 0)
        if rc != 0:
            raise RuntimeError(f"axon_start_nrt_profile rc={rc}")
        try:
            yield
        finally:
            n = lib.axon_stop_nrt_profile(str(output_dir).encode())
            print(f"profile: {n} file(s) written to {output_dir}",
                  file=sys.stderr)

    state = {"h": _hook}
    mod = types.ModuleType("antenv.axon_hooks")
    mod.get_axon_ntff_profile_hook = lambda: state["h"]
    mod.set_axon_ntff_profile_hook = lambda h: state.__setitem__("h", h)
    import antenv
    antenv.axon_hooks = mod
    sys.modules["antenv.axon_hooks"] = mod


def kernel(**inputs):
    x = np.ascontiguousarray(np.asarray(inputs["x"], dtype=np.float32))
    Wq = np.asarray(inputs["Wq"], dtype=np.float32)
    Wk = np.asarray(inputs["Wk"], dtype=np.float32)
    Wv = np.asarray(inputs["Wv"], dtype=np.float32)
    Wo = np.asarray(inputs["Wo"], dtype=np.float32)
    bq = np.asarray(inputs["bq"], dtype=np.float32)
    bk = np.asarray(inputs["bk"], dtype=np.float32)
    bv = np.asarray(inputs["bv"], dtype=np.float32)
    bo = np.asarray(inputs["bo"], dtype=np.float32)

    from concourse import bass_utils

    nc = _build()

    if MM_DTYPE == "bf16":
        import ml_dtypes
        mmd_np = ml_dtypes.bfloat16
    elif MM_DTYPE == "fp16":
        mmd_np = np.float16
    else:
        mmd_np = np.float32

    B = x.shape[0]
    xTs = [np.ascontiguousarray(x[b].T.astype(mmd_np)) for b in range(B)]
    in_maps = []
    for core in range(NCORES):
        b, hg = core // 4, core % 4
        sl = slice(hg * 192, (hg + 1) * 192)
        wq_s, wk_s = Wq[:, sl], Wk[:, sl]
        g0 = wq_s[:, 0:128]
        g1 = wk_s[:, 0:128]
        g2 = np.concatenate([wq_s[:, 128:192], wk_s[:, 128:192]], axis=1)
        wqk_h = (np.stack([g0, g1, g2], 0)
                 .reshape(3, NCH, 128, 128).transpose(2, 0, 1, 3)
                 .reshape(128, 3 * NCH * 128))
        wv_h = (Wv[:, sl].reshape(NCH, 128, 192).transpose(1, 0, 2)
                .reshape(128, NCH * 192))
        wo01_h = Wo[sl, :][0:128, :]
        wo2_h = Wo[sl, :][128:192, :]
        bqk_h = np.stack(
            [bq[sl][0:128], bk[sl][0:128],
             np.concatenate([bq[sl][128:192], bk[sl][128:192]])], axis=1
        )  # [128, 3]
        bv_h = bv[sl].reshape(1, 192)
        in_maps.append({
            "xT": xTs[b],
            "wqk": np.ascontiguousarray(wqk_h.astype(mmd_np)),
            "wv": np.ascontiguousarray(wv_h.astype(mmd_np)),
            "wo01": np.ascontiguousarray(wo01_h.astype(mmd_np)),
            "wo2": np.ascontiguousarray(wo2_h.astype(mmd_np)),
            "bqk": np.ascontiguousarray(bqk_h),
            "bv": np.ascontiguousarray(bv_h),
        })

    trace = bool(os.environ.get("KERNEL_TRACE"))
    if trace:
        _install_trace_hooks()
    res = bass_utils.run_bass_kernel_spmd(
        nc, in_maps, core_ids=list(range(NCORES)), trace=trace
    )
    _cache["last_results"] = res

    out = np.empty((B, T, C), dtype=np.float32)
    for b in range(B):
        acc = res.results[b * 4]["y"].copy()
        for hg in range(1, 4):
            acc += res.results[b * 4 + hg]["y"]
        out[b] = acc.T + bo
    return out



# revision 30
# speedup vs baseline: 1.1531x; 1.1531x over previous
"""Causal self-attention (q/k-swapped variant) Bass kernel for Trainium2.

Problem: B=2, T=2048, C=768, H=12, hs=64.
    k = x@Wk+bk ; q = x@Wq+bq ; v = x@Wv+bv          (per-head split)
    att[b,h,i,j] = (k[b,i,h,:] . q[b,j,h,:]) / 8     (note: k rows, q cols)
    att = softmax(causal-mask(att), axis=j)
    y = (att @ v) @ Wo + bo

Sharding: 8 cores = 2 batches x 4 head-groups (3 heads each).
Each core computes its 3 heads fully (QKV proj -> attention -> partial
output projection); host sums the 4 partial outputs per batch and adds bo.

All on-device score math is done in "transposed score" space: score tiles
have j (softmax axis) on partitions and i on the free dim, so the PV matmul
needs no transposes at all, and the softmax denominator falls out of the PV
matmul via an appended ones-column on V.
"""

import os
import sys

sys.path.insert(0, "/opt/trn_rl_repo")

import numpy as np

T = 2048
C = 768
HS = 64
HPC = 3          # heads per core
NCH = C // 128   # 6 contraction chunks
TB = T // 128    # 16 row blocks
JB = T // 128    # 16 j blocks
NCORES = 8
MM_DTYPE = os.environ.get("KERNEL_MM_DTYPE", "fp16")  # fp16 | bf16 | fp32

_cache = {}


def _segments(lo, hi):
    """Split [lo, hi) at 512 boundaries (PSUM bank / fp32 matmul N limit)."""
    out = []
    s = lo
    while s < hi:
        e = min((s // 512 + 1) * 512, hi)
        out.append((s, e))
        s = e
    return out


def _emit(ctx, tc):
    import concourse.bass as bass
    import concourse.tile as tile  # noqa: F401
    from concourse import mybir
    from concourse.bass import ts
    from concourse.masks import make_upper_triangular

    f32 = mybir.dt.float32
    mmd = {"fp16": mybir.dt.float16, "bf16": mybir.dt.bfloat16,
           "fp32": f32}[MM_DTYPE]  # matmul-input dtype
    nc = tc.nc

    xT = nc.dram_tensor("xT", (C, T), mmd, kind="ExternalInput").ap()
    wqk = nc.dram_tensor("wqk", (128, 3 * NCH * 128), mmd, kind="ExternalInput").ap()
    wv = nc.dram_tensor("wv", (128, NCH * 192), mmd, kind="ExternalInput").ap()
    wo01 = nc.dram_tensor("wo01", (128, C), mmd, kind="ExternalInput").ap()
    wo2 = nc.dram_tensor("wo2", (64, C), mmd, kind="ExternalInput").ap()
    bqk = nc.dram_tensor("bqk", (128, 3), f32, kind="ExternalInput").ap()
    bv = nc.dram_tensor("bv", (1, 192), f32, kind="ExternalInput").ap()
    y = nc.dram_tensor("y", (C, T), f32, kind="ExternalOutput").ap()  # transposed

    consts = ctx.enter_context(tc.tile_pool(name="consts", bufs=1))

    # ---- load inputs (wqk + xT first: they gate the first projections) ----
    wqk_sb = consts.tile([128, 3, NCH, 128], mmd)
    nc.sync.dma_start(wqk_sb[:], wqk.rearrange("p (g k m) -> p g k m", g=3, k=NCH))
    xT_sb = consts.tile([128, NCH, T], mmd)
    for k in range(NCH):
        for q in range(4):  # 4 column slices per chunk: faster first-arrival
            nc.sync.dma_start(xT_sb[:, k, ts(q, 512)],
                              xT[k * 128:(k + 1) * 128, ts(q, 512)])
    wv_sb = consts.tile([128, NCH, 192], mmd)
    nc.sync.dma_start(wv_sb[:], wv.rearrange("p (k m) -> p k m", k=NCH))
    wo01_sb = consts.tile([128, C], mmd)
    nc.sync.dma_start(wo01_sb[:], wo01)
    wo2_sb = consts.tile([64, C], mmd)
    nc.sync.dma_start(wo2_sb[:], wo2)
    bqk_sb = consts.tile([128, 3], f32)       # per-partition bias per QK group
    nc.sync.dma_start(bqk_sb[:], bqk)
    bvb_sb = consts.tile([128, 192], f32)     # bv broadcast across partitions
    nc.sync.dma_start(bvb_sb[:], bv.to_broadcast((128, 192)))

    scratch = consts.tile([128, 512], mmd)
    nc.vector.memset(scratch[:], 0.0)
    ones64 = consts.tile([1, 64], f32)
    nc.vector.memset(ones64[:], 1.0)
    trimask = consts.tile([128, 128], mmd)
    make_upper_triangular(nc, trimask[:], val=1.0, diag=True)

    V_aug = consts.tile([128, TB, HPC * 65], mmd)
    for h in range(HPC):
        nc.vector.memset(V_aug[:, :, h * 65 + 64:h * 65 + 65], 1.0)

    QK_sb = consts.tile([128, 3, T], mmd)     # g0=Q(h0,h1) g1=K(h0,h1) g2=[Q(h2)|K(h2)]
    KT2_sb = consts.tile([64, T], mmd)        # K(h2) shifted to base partition 0
    AT01_sb = consts.tile([128, T], mmd)      # normalized attn out, h0 rows 0:64, h1 64:128
    AT2_sb = consts.tile([64, T], mmd)        # normalized attn out, h2

    # ---- single fused pipeline ----
    # PSUM: psP (proj/outproj, 2 banks) + psS (scores, 4) + psO (Onum, 2) = 8
    psP = ctx.enter_context(tc.tile_pool(name="psP", bufs=2, space="PSUM"))
    psS = ctx.enter_context(tc.tile_pool(name="psS", bufs=2, space="PSUM"))
    psO = ctx.enter_context(tc.tile_pool(name="psO", bufs=1, space="PSUM"))
    sbE = ctx.enter_context(tc.tile_pool(name="E", bufs=5))
    sbATn = ctx.enter_context(tc.tile_pool(name="ATn", bufs=2))
    sbRZ = ctx.enter_context(tc.tile_pool(name="RZ", bufs=2))
    sbY = ctx.enter_context(tc.tile_pool(name="Y", bufs=4))

    # PE warm-up (keeps HAM at full clock while inputs stream in) + exp
    # table pre-load
    for _ in range(28):
        warm = psP.tile([128, 512], f32, tag="p")
        nc.tensor.matmul(warm[:], lhsT=scratch[:, 0:128], rhs=scratch[:],
                         start=True, stop=True, skip_group_check=True)
    edum = sbE.tile([128, 1024], mmd, tag="edum")
    nc.scalar.activation(edum[:, 0:512], scratch[:],
                         mybir.ActivationFunctionType.Exp, scale=0.125)

    def qk_mm(g, it):
        ps = psP.tile([128, 512], f32, tag="p")
        for k in range(NCH):
            nc.tensor.matmul(ps[:], lhsT=wqk_sb[:, g, k, :],
                             rhs=xT_sb[:, k, ts(it, 512)],
                             start=(k == 0), stop=(k == NCH - 1))
        return ps

    def qk_epi(ps, g, it):
        nc.vector.tensor_add(QK_sb[:, g, ts(it, 512)], ps[:],
                             bqk_sb[:, g:g + 1].to_broadcast((128, 512)))

    def qk_group(g, it):
        qk_epi(qk_mm(g, it), g, it)

    def v_mm(tb):
        ps = psP.tile([128, 512], f32, tag="p")
        for k in range(NCH):
            nc.tensor.matmul(ps[:, 0:192], lhsT=xT_sb[:, k, ts(tb, 128)],
                             rhs=wv_sb[:, k, :],
                             start=(k == 0), stop=(k == NCH - 1))
        return ps

    def v_epi(ps, tb):
        for h in range(HPC):
            nc.vector.tensor_add(V_aug[:, tb, h * 65:h * 65 + 64],
                                 ps[:, h * 64:(h + 1) * 64],
                                 bvb_sb[:, h * 64:(h + 1) * 64])

    def v_group(tb):
        v_epi(v_mm(tb), tb)

    def op_mm(cb, tt):
        ps = psP.tile([128, 512], f32, tag="p")
        nc.tensor.matmul(ps[:], lhsT=wo01_sb[:, ts(cb, 128)],
                         rhs=AT01_sb[:, ts(tt, 512)], start=True, stop=False)
        nc.tensor.matmul(ps[:], lhsT=wo2_sb[:, ts(cb, 128)],
                         rhs=AT2_sb[:, ts(tt, 512)], start=False, stop=True)
        return ps

    def op_epi(ps, cb, tt):
        ysb = sbY.tile([128, 512], f32)
        nc.vector.tensor_copy(ysb[:], ps[:])
        nc.sync.dma_start(y[cb * 128:(cb + 1) * 128, tt * 512:(tt + 1) * 512],
                          ysb[:])

    def kt2_shift(lo, hi):
        nc.sync.dma_start(KT2_sb[:, lo:hi], QK_sb[64:128, 2, lo:hi])

    # pre-phase: K^T cols 0:1023 of h0/h1, Q^T cols for jb 0-3, V blocks
    # 0-5 (unit h0's trailing PVs consume V(0..7) by its last chunk, and
    # deferred-epilogue fillers deliver at most v6/v7 in time); everything
    # else weaves into the chunk stream as PE filler.
    for it in range(2):
        qk_group(1, it)
    qk_group(0, 0)
    for tb in range(6):
        v_group(tb)

    from collections import deque
    # Filler groups are (mm_emitter, epi_maker) pairs. The mm stage runs PE
    # matmuls; the epilogue (a DVE bias-add / PSUM->SBUF copy) is deferred
    # two pops so it never enqueues on DVE while its matmuls are still in
    # flight (head-of-line blocking on the strict-FIFO DVE queue).
    # Emission-order invariants (Tile deps follow emission order):
    #  - qk(2,*)+kt2_shift before the half0-h2 unit (its STs read g2/KT2);
    #    they sit at pops 10-14, i.e. during half0-h1 -> barrier is a no-op.
    #  - qk(1,2..3) epi before half1's first ST (chunk 24): pops 3/5 +2.
    #  - v(tb) epi before the first PV reading V_aug[:,tb]: v8-15 pop at
    #    chunks 15-22, first consumer is half1-h1's PV(8) around chunk 35.
    def G(mm, epi=None):
        return (mm, epi)

    def qk_G(g, it):
        return G(lambda: qk_mm(g, it), lambda ps: qk_epi(ps, g, it))

    def v_G(tb):
        return G(lambda: v_mm(tb), lambda ps: v_epi(ps, tb))

    # kt2 shifts sit >=2 items after the qk(2,*) epis they read, so the
    # due-discipline of the epi backlog emits those adds first
    pre_fillers = deque(
        [qk_G(0, 1), v_G(6), v_G(7), qk_G(1, 2), qk_G(1, 3),
         qk_G(0, 2), qk_G(0, 3), qk_G(2, 0), qk_G(2, 1),
         qk_G(2, 2), qk_G(2, 3),
         G(lambda: (kt2_shift(0, 1024), None)[1]),
         G(lambda: (kt2_shift(1024, 2048), None)[1])]
    )
    v_late = deque([v_G(tb) for tb in range(8, TB)])
    op_fillers = deque()
    epi_backlog = deque()   # (due_popcount, thunk)
    popcnt = [0]

    def pop_filler():
        popcnt[0] += 1
        if epi_backlog and epi_backlog[0][0] <= popcnt[0]:
            epi_backlog.popleft()[1]()
            return True
        for q in (pre_fillers, v_late, op_fillers):
            if q:
                mm, epi = q.popleft()
                ps = mm()
                if epi is not None:
                    epi_backlog.append((popcnt[0] + 2, lambda: epi(ps)))
                return True
        if epi_backlog:
            epi_backlog.popleft()[1]()
            return True
        return False

    def flush_fillers(queues):
        for q in queues:
            while q:
                pop_filler()
        while epi_backlog:
            epi_backlog.popleft()[1]()

    # per-head (lhsT=Q^T, rhs=K^T) access patterns; partition bases match
    heads = [
        (QK_sb[0:64, 0, :], QK_sb[0:64, 1, :]),
        (QK_sb[64:128, 0, :], QK_sb[64:128, 1, :]),
        (QK_sb[0:64, 2, :], KT2_sb[:, :]),
    ]

    HW = 1024  # i-window per (half, head) unit
    # half1 runs h1 first so its AT-shift DMA clears long before the tail,
    # and ends on h0 whose normalization chain has no trailing shift.
    head_order = [(0, 1, 2), (1, 2, 0)]
    for half in range(T // HW):
        c0 = HW * half
        njb = (c0 + HW) // 128
        for h in head_order[half]:
            if h == 2 and (pre_fillers or epi_backlog):
                # h2 reads g2/KT2: force their writers (and any pending
                # epilogues) out now; normally a no-op by pop scheduling
                flush_fillers((pre_fillers,))
            QT, KT = heads[h]
            Onum = psO.tile([65, HW], f32)

            def emit_pv(jb, E2, par, lo):
                for a, b in _segments(lo, c0 + HW):
                    nc.tensor.matmul(Onum[:, a - c0:b - c0],
                                     lhsT=V_aug[:, jb, h * 65:(h + 1) * 65],
                                     rhs=E2[:, par, a - c0:b - c0],
                                     start=(jb == 0),
                                     stop=(jb == min(4 * (a // 512) + 3,
                                                     njb - 1)),
                                     skip_group_check=True)

            pending = []
            for jb in range(njb):
                i0 = 128 * jb
                lo = max(c0, i0)
                S = psS.tile([128, HW], f32)
                for a, b in _segments(lo, c0 + HW):
                    nc.tensor.matmul(S[:, a - c0:b - c0],
                                     lhsT=QT[:, ts(jb, 128)],
                                     rhs=KT[:, a:b], start=True, stop=True)
                E = sbE.tile([128, 1, HW], mmd)
                nc.scalar.activation(E[:, 0, lo - c0:], S[:, lo - c0:],
                                     mybir.ActivationFunctionType.Exp,
                                     scale=0.125)
                if lo == i0:  # window containing the diagonal block
                    r = i0 - c0
                    nc.vector.tensor_mul(E[:, 0, r:r + 128], E[:, 0, r:r + 128],
                                         trimask[:])
                if not pop_filler():
                    # one dummy matmul per filler miss keeps the PE activity
                    # monitor from re-throttling the clock mid-attention
                    warm = psP.tile([128, 512], f32, tag="p")
                    nc.tensor.matmul(warm[:], lhsT=scratch[:, 0:128],
                                     rhs=scratch[:], start=True, stop=True,
                                     skip_group_check=True)
                pending.append((jb, E, 0, lo))
                if len(pending) > 3:  # PV trails ST by 3 chunks
                    emit_pv(*pending.pop(0))
            for item in pending:
                emit_pv(*item)

            # row 64 of Onum is Z, on one partition. DMA-reshape it straight
            # from PSUM to [128, HW/128] (overlapping the ATn copy) for a
            # parallel DVE reciprocal, fold back, then gpsimd replicates 1/Z
            # across 64 partitions for the divide.
            ATn = sbATn.tile([65, HW], f32)
            nc.vector.tensor_copy(ATn[:], Onum[:])
            z16 = sbRZ.tile([128, HW // 128], f32, tag="z16")
            nc.sync.dma_start(z16[:], ATn[64:65, :])
            r16 = sbRZ.tile([128, HW // 128], f32, tag="r16")
            nc.vector.reciprocal(r16[:], z16[:])
            rz1 = sbRZ.tile([1, HW], f32, tag="rz1")
            nc.sync.dma_start(rz1[:], r16[:])
            rzb = sbRZ.tile([64, HW], f32, tag="rzb")
            nc.gpsimd.partition_broadcast(rzb[:], rz1[:], channels=64)
            if h == 0:
                nc.vector.tensor_mul(AT01_sb[0:64, c0:c0 + HW], ATn[0:64, :],
                                     rzb[:])
            elif h == 2:
                nc.vector.tensor_mul(AT2_sb[:, c0:c0 + HW], ATn[0:64, :],
                                     rzb[:])
            else:
                # h1's rows live at partitions 64:128 of AT01: normalize into
                # a scratch tile, then partition-shift via SBUF-to-SBUF DMA.
                ATsh = sbRZ.tile([64, HW], mmd, tag="atsh")
                nc.vector.tensor_mul(ATsh[:], ATn[0:64, :], rzb[:])
                nc.sync.dma_start(AT01_sb[64:128, c0:c0 + HW], ATsh[:])

        # all heads done for this half: its output columns can project out;
        # groups run as fillers inside the next half (or drain at the end)
        for cb in range(NCH):
            for tt in range(c0 // 512, (c0 + HW) // 512):
                op_fillers.append(
                    G(lambda cb=cb, tt=tt: op_mm(cb, tt),
                      lambda ps, cb=cb, tt=tt: op_epi(ps, cb, tt)))

    # drain remaining fillers (the last half's output projection); a few
    # dummies bridge the last normalization chain so the PE stays warm
    for _ in range(8):
        warm = psP.tile([128, 512], f32, tag="p")
        nc.tensor.matmul(warm[:], lhsT=scratch[:, 0:128], rhs=scratch[:],
                         start=True, stop=True, skip_group_check=True)
    flush_fillers((pre_fillers, v_late, op_fillers))


def _build():
    if "nc" in _cache:
        return _cache["nc"]
    from contextlib import ExitStack

    import concourse.tile as tile
    from concourse import bacc

    nc = bacc.Bacc("TRN2", target_bir_lowering=False, debug=False,
                   num_devices=NCORES)
    with tile.TileContext(nc) as tc:
        with ExitStack() as ctx:
            _emit(ctx, tc)
    nc.compile()
    _cache["nc"] = nc
    return nc


def _install_trace_hooks():
    """Make trace=True work in this container: shim the missing
    antenv.axon_hooks NTFF-profile hook (ctypes into libaxon_pjrt.so) and
    skip the S3 artifact upload."""
    import contextlib
    import ctypes
    import types

    import concourse.bass_utils as bu

    bu.upload_artifacts = lambda tmpdir: tmpdir
    try:
        from antenv.axon_hooks import get_axon_ntff_profile_hook  # noqa: F401
        return
    except ImportError:
        pass

    so_path = "/opt/axon/libaxon_pjrt.so"
    if not os.path.exists(so_path):
        return
    lib = ctypes.CDLL(so_path)
    if not hasattr(lib, "axon_start_nrt_profile"):
        return
    lib.axon_start_nrt_profile.argtypes = [
        ctypes.POINTER(ctypes.c_int64), ctypes.c_size_t,
    ]
    lib.axon_start_nrt_profile.restype = ctypes.c_int64
    lib.axon_stop_nrt_profile.argtypes = [ctypes.c_char_p]
    lib.axon_stop_nrt_profile.restype = ctypes.c_int64

    @contextlib.contextmanager
    def _hook(output_dir, device_ids):
        import jax
        jax.devices()
        if device_ids:
            ids = (ctypes.c_int64 * len(device_ids))(*device_ids)
            rc = lib.axon_start_nrt_profile(ids, len(device_ids))
        else:
            rc = lib.axon_start_nrt_profile(None, 0)
        if rc != 0:
            raise RuntimeError(f"axon_start_nrt_profile rc={rc}")
        try:
            yield
        finally:
            n = lib.axon_stop_nrt_profile(str(output_dir).encode())
            print(f"profile: {n} file(s) written to {output_dir}",
                  file=sys.stderr)

    state = {"h": _hook}
    mod = types.ModuleType("antenv.axon_hooks")
    mod.get_axon_ntff_profile_hook = lambda: state["h"]
    mod.set_axon_ntff_profile_hook = lambda h: state.__setitem__("h", h)
    import antenv
    antenv.axon_hooks = mod
    sys.modules["antenv.axon_hooks"] = mod


def kernel(**inputs):
    x = np.ascontiguousarray(np.asarray(inputs["x"], dtype=np.float32))
    Wq = np.asarray(inputs["Wq"], dtype=np.float32)
    Wk = np.asarray(inputs["Wk"], dtype=np.float32)
    Wv = np.asarray(inputs["Wv"], dtype=np.float32)
    Wo = np.asarray(inputs["Wo"], dtype=np.float32)
    bq = np.asarray(inputs["bq"], dtype=np.float32)
    bk = np.asarray(inputs["bk"], dtype=np.float32)
    bv = np.asarray(inputs["bv"], dtype=np.float32)
    bo = np.asarray(inputs["bo"], dtype=np.float32)

    from concourse import bass_utils

    nc = _build()

    if MM_DTYPE == "bf16":
        import ml_dtypes
        mmd_np = ml_dtypes.bfloat16
    elif MM_DTYPE == "fp16":
        mmd_np = np.float16
    else:
        mmd_np = np.float32

    B = x.shape[0]
    xTs = [np.ascontiguousarray(x[b].T.astype(mmd_np)) for b in range(B)]
    in_maps = []
    for core in range(NCORES):
        b, hg = core // 4, core % 4
        sl = slice(hg * 192, (hg + 1) * 192)
        wq_s, wk_s = Wq[:, sl], Wk[:, sl]
        g0 = wq_s[:, 0:128]
        g1 = wk_s[:, 0:128]
        g2 = np.concatenate([wq_s[:, 128:192], wk_s[:, 128:192]], axis=1)
        wqk_h = (np.stack([g0, g1, g2], 0)
                 .reshape(3, NCH, 128, 128).transpose(2, 0, 1, 3)
                 .reshape(128, 3 * NCH * 128))
        wv_h = (Wv[:, sl].reshape(NCH, 128, 192).transpose(1, 0, 2)
                .reshape(128, NCH * 192))
        wo01_h = Wo[sl, :][0:128, :]
        wo2_h = Wo[sl, :][128:192, :]
        bqk_h = np.stack(
            [bq[sl][0:128], bk[sl][0:128],
             np.concatenate([bq[sl][128:192], bk[sl][128:192]])], axis=1
        )  # [128, 3]
        bv_h = bv[sl].reshape(1, 192)
        in_maps.append({
            "xT": xTs[b],
            "wqk": np.ascontiguousarray(wqk_h.astype(mmd_np)),
            "wv": np.ascontiguousarray(wv_h.astype(mmd_np)),
            "wo01": np.ascontiguousarray(wo01_h.astype(mmd_np)),
            "wo2": np.ascontiguousarray(wo2_h.astype(mmd_np)),
            "bqk": np.ascontiguousarray(bqk_h),
            "bv": np.ascontiguousarray(bv_h),
        })

    trace = bool(os.environ.get("KERNEL_TRACE"))
    if trace:
        _install_trace_hooks()
    res = bass_utils.run_bass_kernel_spmd(
        nc, in_maps, core_ids=list(range(NCORES)), trace=trace
    )
    _cache["last_results"] = res

    out = np.empty((B, T, C), dtype=np.float32)
    for b in range(B):
        acc = res.results[b * 4]["y"].copy()
        for hg in range(1, 4):
            acc += res.results[b * 4 + hg]["y"]
        out[b] = acc.T + bo
    return out



# revision 35
# speedup vs baseline: 1.2180x; 1.0563x over previous
"""Causal self-attention (q/k-swapped variant) Bass kernel for Trainium2.

Problem: B=2, T=2048, C=768, H=12, hs=64.
    k = x@Wk+bk ; q = x@Wq+bq ; v = x@Wv+bv          (per-head split)
    att[b,h,i,j] = (k[b,i,h,:] . q[b,j,h,:]) / 8     (note: k rows, q cols)
    att = softmax(causal-mask(att), axis=j)
    y = (att @ v) @ Wo + bo

Sharding: 8 cores = 2 batches x 4 head-groups (3 heads each).
Each core computes its 3 heads fully (QKV proj -> attention -> partial
output projection); host sums the 4 partial outputs per batch and adds bo.

All on-device score math is done in "transposed score" space: score tiles
have j (softmax axis) on partitions and i on the free dim, so the PV matmul
needs no transposes at all, and the softmax denominator falls out of the PV
matmul via an appended ones-column on V.
"""

import os
import sys

sys.path.insert(0, "/opt/trn_rl_repo")

import numpy as np

T = 2048
C = 768
HS = 64
HPC = 3          # heads per core
NCH = C // 128   # 6 contraction chunks
TB = T // 128    # 16 row blocks
JB = T // 128    # 16 j blocks
NCORES = 8
MM_DTYPE = os.environ.get("KERNEL_MM_DTYPE", "fp16")  # fp16 | bf16 | fp32

_cache = {}


def _segments(lo, hi):
    """Split [lo, hi) at 512 boundaries (PSUM bank / fp32 matmul N limit)."""
    out = []
    s = lo
    while s < hi:
        e = min((s // 512 + 1) * 512, hi)
        out.append((s, e))
        s = e
    return out


def _emit(ctx, tc):
    import concourse.bass as bass
    import concourse.tile as tile  # noqa: F401
    from concourse import mybir
    from concourse.bass import ts
    from concourse.masks import make_upper_triangular

    f32 = mybir.dt.float32
    mmd = {"fp16": mybir.dt.float16, "bf16": mybir.dt.bfloat16,
           "fp32": f32}[MM_DTYPE]  # matmul-input dtype
    nc = tc.nc

    xT = nc.dram_tensor("xT", (C, T), mmd, kind="ExternalInput").ap()
    wqk = nc.dram_tensor("wqk", (128, 3 * NCH * 128), mmd, kind="ExternalInput").ap()
    wv = nc.dram_tensor("wv", (128, NCH * 192), mmd, kind="ExternalInput").ap()
    wo01 = nc.dram_tensor("wo01", (128, C), mmd, kind="ExternalInput").ap()
    wo2 = nc.dram_tensor("wo2", (64, C), mmd, kind="ExternalInput").ap()
    bqk = nc.dram_tensor("bqk", (128, 3), f32, kind="ExternalInput").ap()
    bv = nc.dram_tensor("bv", (1, 192), f32, kind="ExternalInput").ap()
    y = nc.dram_tensor("y", (C, T), f32, kind="ExternalOutput").ap()  # transposed

    consts = ctx.enter_context(tc.tile_pool(name="consts", bufs=1))

    # ---- load inputs (wqk + xT first: they gate the first projections) ----
    wqk_sb = consts.tile([128, 3, NCH, 128], mmd)
    nc.sync.dma_start(wqk_sb[:], wqk.rearrange("p (g k m) -> p g k m", g=3, k=NCH))
    xT_sb = consts.tile([128, NCH, T], mmd)
    for k in range(NCH):
        nc.sync.dma_start(xT_sb[:, k, :], xT[k * 128:(k + 1) * 128, :])
    wv_sb = consts.tile([128, NCH, 192], mmd)
    nc.sync.dma_start(wv_sb[:], wv.rearrange("p (k m) -> p k m", k=NCH))
    wo01_sb = consts.tile([128, C], mmd)
    nc.sync.dma_start(wo01_sb[:], wo01)
    wo2_sb = consts.tile([64, C], mmd)
    nc.sync.dma_start(wo2_sb[:], wo2)
    bqk_sb = consts.tile([128, 3], f32)       # per-partition bias per QK group
    nc.sync.dma_start(bqk_sb[:], bqk)
    bvb_sb = consts.tile([128, 192], f32)     # bv broadcast across partitions
    nc.sync.dma_start(bvb_sb[:], bv.to_broadcast((128, 192)))

    scratch = consts.tile([128, 512], mmd)
    nc.vector.memset(scratch[:], 0.0)
    ones64 = consts.tile([1, 64], f32)
    nc.vector.memset(ones64[:], 1.0)
    trimask = consts.tile([128, 128], mmd)
    make_upper_triangular(nc, trimask[:], val=1.0, diag=True)

    V_aug = consts.tile([128, TB, HPC * 65], mmd)
    for h in range(HPC):
        nc.vector.memset(V_aug[:, :, h * 65 + 64:h * 65 + 65], 1.0)

    QK_sb = consts.tile([128, 3, T], mmd)     # g0=Q(h0,h1) g1=K(h0,h1) g2=[Q(h2)|K(h2)]
    KT2_sb = consts.tile([64, T], mmd)        # K(h2) shifted to base partition 0
    AT01_sb = consts.tile([128, T], mmd)      # normalized attn out, h0 rows 0:64, h1 64:128
    AT2_sb = consts.tile([64, T], mmd)        # normalized attn out, h2

    # ---- single fused pipeline ----
    # PSUM: psP (proj/outproj, 2 banks) + psS (scores, 4) + psO (Onum, 2) = 8
    psP = ctx.enter_context(tc.tile_pool(name="psP", bufs=2, space="PSUM"))
    psS = ctx.enter_context(tc.tile_pool(name="psS", bufs=2, space="PSUM"))
    psO = ctx.enter_context(tc.tile_pool(name="psO", bufs=1, space="PSUM"))
    sbE = ctx.enter_context(tc.tile_pool(name="E", bufs=5))
    sbATn = ctx.enter_context(tc.tile_pool(name="ATn", bufs=2))
    sbRZ = ctx.enter_context(tc.tile_pool(name="RZ", bufs=2))
    sbY = ctx.enter_context(tc.tile_pool(name="Y", bufs=4))

    # PE warm-up (keeps HAM at full clock while inputs stream in) + exp
    # table pre-load
    for _ in range(28):
        warm = psP.tile([128, 512], f32, tag="p")
        nc.tensor.matmul(warm[:], lhsT=scratch[:, 0:128], rhs=scratch[:],
                         start=True, stop=True, skip_group_check=True)
    edum = sbE.tile([128, 1024], mmd)
    nc.scalar.activation(edum[:, 0:512], scratch[:],
                         mybir.ActivationFunctionType.Exp, scale=0.125)

    def qk_mm(g, it):
        ps = psP.tile([128, 512], f32, tag="p")
        for k in range(NCH):
            nc.tensor.matmul(ps[:], lhsT=wqk_sb[:, g, k, :],
                             rhs=xT_sb[:, k, ts(it, 512)],
                             start=(k == 0), stop=(k == NCH - 1))
        return ps

    def qk_epi(ps, g, it):
        nc.vector.tensor_add(QK_sb[:, g, ts(it, 512)], ps[:],
                             bqk_sb[:, g:g + 1].to_broadcast((128, 512)))

    def qk_group(g, it):
        qk_epi(qk_mm(g, it), g, it)

    def v_mm(tb):
        ps = psP.tile([128, 512], f32, tag="p")
        for k in range(NCH):
            nc.tensor.matmul(ps[:, 0:192], lhsT=xT_sb[:, k, ts(tb, 128)],
                             rhs=wv_sb[:, k, :],
                             start=(k == 0), stop=(k == NCH - 1))
        return ps

    def v_epi(ps, tb):
        for h in range(HPC):
            nc.vector.tensor_add(V_aug[:, tb, h * 65:h * 65 + 64],
                                 ps[:, h * 64:(h + 1) * 64],
                                 bvb_sb[:, h * 64:(h + 1) * 64])

    def v_group(tb):
        v_epi(v_mm(tb), tb)

    def op_mm(cb, tt):
        ps = psP.tile([128, 512], f32, tag="p")
        nc.tensor.matmul(ps[:], lhsT=wo01_sb[:, ts(cb, 128)],
                         rhs=AT01_sb[:, ts(tt, 512)], start=True, stop=False)
        nc.tensor.matmul(ps[:], lhsT=wo2_sb[:, ts(cb, 128)],
                         rhs=AT2_sb[:, ts(tt, 512)], start=False, stop=True)
        return ps

    def op_epi(ps, cb, tt):
        ysb = sbY.tile([128, 512], f32)
        nc.vector.tensor_copy(ysb[:], ps[:])
        nc.sync.dma_start(y[cb * 128:(cb + 1) * 128, tt * 512:(tt + 1) * 512],
                          ysb[:])

    def kt2_shift(lo, hi):
        nc.sync.dma_start(KT2_sb[:, lo:hi], QK_sb[64:128, 2, lo:hi])

    # pre-phase: the minimum the first unit needs before its first chunk;
    # the rest of the first unit's inputs arrive as inline-epilogue fillers
    # popped from inside its chunk loop (deadlines: qk(0,1) by pop 2, v(tb)
    # by pop tb+1).
    for it in range(2):
        qk_group(1, it)
    qk_group(0, 0)
    v_group(0)

    from collections import deque
    # Filler groups are (mm_emitter, epi_maker) pairs. The mm stage runs PE
    # matmuls; the epilogue (a DVE bias-add / PSUM->SBUF copy) is deferred
    # two pops so it never enqueues on DVE while its matmuls are still in
    # flight (head-of-line blocking on the strict-FIFO DVE queue).
    # Emission-order invariants (Tile deps follow emission order):
    #  - qk(2,*)+kt2_shift before the half0-h2 unit (its STs read g2/KT2);
    #    they sit at pops 10-14, i.e. during half0-h1 -> barrier is a no-op.
    #  - qk(1,2..3) epi before half1's first ST (chunk 24): pops 3/5 +2.
    #  - v(tb) epi before the first PV reading V_aug[:,tb]: v8-15 pop at
    #    chunks 15-22, first consumer is half1-h1's PV(8) around chunk 35.
    def G(mm, epi=None):
        return (mm, epi)

    def qk_G(g, it):
        return G(lambda: qk_mm(g, it), lambda ps: qk_epi(ps, g, it))

    def v_G(tb):
        return G(lambda: v_mm(tb), lambda ps: v_epi(ps, tb))

    # first-unit fillers run with INLINE epilogues (one full group per pop)
    # so the deadline-dense v1..v7 bias-adds land before the PVs that read
    # them; DVE head-of-line cost is negligible while the pipe fills.
    first_fillers = deque(
        [G(lambda: (qk_group(0, 1), None)[1])]
        + [G(lambda tb=tb: (v_group(tb), None)[1]) for tb in range(1, 8)]
    )
    # kt2 shifts sit >=2 items after the qk(2,*) epis they read, so the
    # due-discipline of the epi backlog emits those adds first
    pre_fillers = deque(
        [qk_G(1, 2), qk_G(1, 3), qk_G(2, 0), qk_G(2, 1),
         qk_G(2, 2), qk_G(2, 3),
         G(lambda: (kt2_shift(0, 1024), None)[1]),
         G(lambda: (kt2_shift(1024, 2048), None)[1])]
    )
    qk0_late = deque([qk_G(0, 2), qk_G(0, 3)])
    v_late = deque([v_G(tb) for tb in range(8, TB)])
    op_fillers = deque()
    epi_backlog = deque()   # (due_popcount, thunk)
    popcnt = [0]

    def pop_filler():
        popcnt[0] += 1
        if epi_backlog and epi_backlog[0][0] <= popcnt[0]:
            epi_backlog.popleft()[1]()
            return True
        for q in (first_fillers, pre_fillers, qk0_late, v_late, op_fillers):
            if q:
                mm, epi = q.popleft()
                ps = mm()
                if epi is not None:
                    epi_backlog.append((popcnt[0] + 2, lambda: epi(ps)))
                return True
        if epi_backlog:
            epi_backlog.popleft()[1]()
            return True
        return False

    def flush_fillers(queues):
        for q in queues:
            while q:
                pop_filler()
        while epi_backlog:
            epi_backlog.popleft()[1]()

    # per-head (lhsT=Q^T, rhs=K^T) access patterns; partition bases match
    heads = [
        (QK_sb[0:64, 0, :], QK_sb[0:64, 1, :]),
        (QK_sb[64:128, 0, :], QK_sb[64:128, 1, :]),
        (QK_sb[0:64, 2, :], KT2_sb[:, :]),
    ]

    HW = 1024  # i-window per (half, head) unit
    # half1 runs h1 first so its AT-shift DMA clears long before the tail,
    # and ends on h0 whose normalization chain has no trailing shift.
    head_order = [(0, 1, 2), (1, 2, 0)]
    for half in range(T // HW):
        c0 = HW * half
        njb = (c0 + HW) // 128
        for h in head_order[half]:
            if h == 2 and (pre_fillers or epi_backlog):
                # h2 reads g2/KT2: force their writers (and any pending
                # epilogues) out now; normally a no-op by pop scheduling
                flush_fillers((pre_fillers,))
            QT, KT = heads[h]
            Onum = psO.tile([65, HW], f32)

            def emit_pv(jb, E, lo):
                for a, b in _segments(lo, c0 + HW):
                    nc.tensor.matmul(Onum[:, a - c0:b - c0],
                                     lhsT=V_aug[:, jb, h * 65:(h + 1) * 65],
                                     rhs=E[:, a - c0:b - c0],
                                     start=(jb == 0),
                                     stop=(jb == min(4 * (a // 512) + 3,
                                                     njb - 1)),
                                     skip_group_check=True)

            # the last unit's first 512-col segment finishes accumulating
            # at jb = 4*(c0//512)+3: launch its normalization chain early so
            # its output projection overlaps the rest of the unit
            last_unit = (half == 1 and h == head_order[1][-1])
            segA_last_jb = min(4 * (c0 // 512) + 3, njb - 1)

            def emit_chain_seg(s0, s1):
                """normalize Onum[:, s0:s1] -> AT01[0:64, c0+s0:c0+s1]"""
                w = s1 - s0
                ATs = sbATn.tile([65, 512], f32, tag="a512")
                nc.vector.tensor_copy(ATs[:, 0:w], Onum[:, s0:s1])
                z16s = sbRZ.tile([128, 512 // 128], f32, tag="z16s")
                nc.sync.dma_start(z16s[:], ATs[64:65, 0:w])
                r16s = sbRZ.tile([128, 512 // 128], f32, tag="r16s")
                nc.vector.reciprocal(r16s[:], z16s[:])
                rz1s = sbRZ.tile([1, 512], f32, tag="rz1s")
                nc.sync.dma_start(rz1s[:], r16s[:])
                rzbs = sbRZ.tile([64, 512], f32, tag="rzbs")
                nc.gpsimd.partition_broadcast(rzbs[:], rz1s[:], channels=64)
                nc.vector.tensor_mul(AT01_sb[0:64, c0 + s0:c0 + s1],
                                     ATs[0:64, 0:w], rzbs[:])

            pending = []
            for jb in range(njb):
                i0 = 128 * jb
                lo = max(c0, i0)
                S = psS.tile([128, HW], f32)
                for a, b in _segments(lo, c0 + HW):
                    nc.tensor.matmul(S[:, a - c0:b - c0],
                                     lhsT=QT[:, ts(jb, 128)],
                                     rhs=KT[:, a:b], start=True, stop=True)
                E = sbE.tile([128, HW], mmd)
                nc.scalar.activation(E[:, lo - c0:], S[:, lo - c0:],
                                     mybir.ActivationFunctionType.Exp,
                                     scale=0.125)
                if lo == i0:  # window containing the diagonal block
                    r = i0 - c0
                    nc.vector.tensor_mul(E[:, r:r + 128], E[:, r:r + 128],
                                         trimask[:])
                if not pop_filler():
                    # one dummy matmul per filler miss keeps the PE activity
                    # monitor from re-throttling the clock mid-attention
                    warm = psP.tile([128, 512], f32, tag="p")
                    nc.tensor.matmul(warm[:], lhsT=scratch[:, 0:128],
                                     rhs=scratch[:], start=True, stop=True,
                                     skip_group_check=True)
                pending.append((jb, E, lo))
                if len(pending) > 3:  # PV trails ST by 3 chunks
                    done = pending.pop(0)
                    emit_pv(*done)
                    if last_unit and done[0] == segA_last_jb:
                        emit_chain_seg(0, 512)
            for item in pending:
                emit_pv(*item)
                if last_unit and item[0] == segA_last_jb:
                    emit_chain_seg(0, 512)

            if last_unit:
                emit_chain_seg(512, HW)
                continue

            # row 64 of Onum is Z, on one partition. DMA-reshape it straight
            # from PSUM to [128, HW/128] (overlapping the ATn copy) for a
            # parallel DVE reciprocal, fold back, then gpsimd replicates 1/Z
            # across 64 partitions for the divide.
            ATn = sbATn.tile([65, HW], f32)
            nc.vector.tensor_copy(ATn[:], Onum[:])
            z16 = sbRZ.tile([128, HW // 128], f32, tag="z16")
            nc.sync.dma_start(z16[:], ATn[64:65, :])
            r16 = sbRZ.tile([128, HW // 128], f32, tag="r16")
            nc.vector.reciprocal(r16[:], z16[:])
            rz1 = sbRZ.tile([1, HW], f32, tag="rz1")
            nc.sync.dma_start(rz1[:], r16[:])
            rzb = sbRZ.tile([64, HW], f32, tag="rzb")
            nc.gpsimd.partition_broadcast(rzb[:], rz1[:], channels=64)
            if h == 0:
                nc.vector.tensor_mul(AT01_sb[0:64, c0:c0 + HW], ATn[0:64, :],
                                     rzb[:])
            elif h == 2:
                nc.vector.tensor_mul(AT2_sb[:, c0:c0 + HW], ATn[0:64, :],
                                     rzb[:])
            else:
                # h1's rows live at partitions 64:128 of AT01: normalize into
                # a scratch tile, then partition-shift via SBUF-to-SBUF DMA.
                ATsh = sbRZ.tile([64, HW], mmd, tag="atsh")
                nc.vector.tensor_mul(ATsh[:], ATn[0:64, :], rzb[:])
                nc.sync.dma_start(AT01_sb[64:128, c0:c0 + HW], ATsh[:])

        # all heads done for this half: its output columns can project out;
        # groups run as fillers inside the next half (or drain at the end)
        for tt in range(c0 // 512, (c0 + HW) // 512):
            for cb in range(NCH):
                op_fillers.append(
                    G(lambda cb=cb, tt=tt: op_mm(cb, tt),
                      lambda ps, cb=cb, tt=tt: op_epi(ps, cb, tt)))

    # drain remaining fillers (the last half's output projection); a few
    # dummies bridge the last normalization chain so the PE stays warm
    for _ in range(8):
        warm = psP.tile([128, 512], f32, tag="p")
        nc.tensor.matmul(warm[:], lhsT=scratch[:, 0:128], rhs=scratch[:],
                         start=True, stop=True, skip_group_check=True)
    flush_fillers((first_fillers, pre_fillers, qk0_late, v_late,
                   op_fillers))


def _build():
    if "nc" in _cache:
        return _cache["nc"]
    from contextlib import ExitStack

    import concourse.tile as tile
    from concourse import bacc

    nc = bacc.Bacc("TRN2", target_bir_lowering=False, debug=False,
                   num_devices=NCORES)
    with tile.TileContext(nc) as tc:
        with ExitStack() as ctx:
            _emit(ctx, tc)
    nc.compile()
    _cache["nc"] = nc
    return nc


def _install_trace_hooks():
    """Make trace=True work in this container: shim the missing
    antenv.axon_hooks NTFF-profile hook (ctypes into libaxon_pjrt.so) and
    skip the S3 artifact upload."""
    import contextlib
    import ctypes
    import types

    import concourse.bass_utils as bu

    bu.upload_artifacts = lambda tmpdir: tmpdir
    try:
        from antenv.axon_hooks import get_axon_ntff_profile_hook  # noqa: F401
        return
    except ImportError:
        pass

    so_path = "/opt/axon/libaxon_pjrt.so"
    if not os.path.exists(so_path):
        return
    lib = ctypes.CDLL(so_path)
    if not hasattr(lib, "axon_start_nrt_profile"):
        return
    lib.axon_start_nrt_profile.argtypes = [
        ctypes.POINTER(ctypes.c_int64), ctypes.c_size_t,
    ]
    lib.axon_start_nrt_profile.restype = ctypes.c_int64
    lib.axon_stop_nrt_profile.argtypes = [ctypes.c_char_p]
    lib.axon_stop_nrt_profile.restype = ctypes.c_int64

    @contextlib.contextmanager
    def _hook(output_dir, device_ids):
        import jax
        jax.devices()
        if device_ids:
            ids = (ctypes.c_int64 * len(device_ids))(*device_ids)
            rc = lib.axon_start_nrt_profile(ids, len(device_ids))
        else:
            rc = lib.axon_start_nrt_profile(None, 0)
        if rc != 0:
            raise RuntimeError(f"axon_start_nrt_profile rc={rc}")
        try:
            yield
        finally:
            n = lib.axon_stop_nrt_profile(str(output_dir).encode())
            print(f"profile: {n} file(s) written to {output_dir}",
                  file=sys.stderr)

    state = {"h": _hook}
    mod = types.ModuleType("antenv.axon_hooks")
    mod.get_axon_ntff_profile_hook = lambda: state["h"]
    mod.set_axon_ntff_profile_hook = lambda h: state.__setitem__("h", h)
    import antenv
    antenv.axon_hooks = mod
    sys.modules["antenv.axon_hooks"] = mod


def kernel(**inputs):
    x = np.ascontiguousarray(np.asarray(inputs["x"], dtype=np.float32))
    Wq = np.asarray(inputs["Wq"], dtype=np.float32)
    Wk = np.asarray(inputs["Wk"], dtype=np.float32)
    Wv = np.asarray(inputs["Wv"], dtype=np.float32)
    Wo = np.asarray(inputs["Wo"], dtype=np.float32)
    bq = np.asarray(inputs["bq"], dtype=np.float32)
    bk = np.asarray(inputs["bk"], dtype=np.float32)
    bv = np.asarray(inputs["bv"], dtype=np.float32)
    bo = np.asarray(inputs["bo"], dtype=np.float32)

    from concourse import bass_utils

    nc = _build()

    if MM_DTYPE == "bf16":
        import ml_dtypes
        mmd_np = ml_dtypes.bfloat16
    elif MM_DTYPE == "fp16":
        mmd_np = np.float16
    else:
        mmd_np = np.float32

    B = x.shape[0]
    xTs = [np.ascontiguousarray(x[b].T.astype(mmd_np)) for b in range(B)]
    in_maps = []
    for core in range(NCORES):
        b, hg = core // 4, core % 4
        sl = slice(hg * 192, (hg + 1) * 192)
        wq_s, wk_s = Wq[:, sl], Wk[:, sl]
        g0 = wq_s[:, 0:128]
        g1 = wk_s[:, 0:128]
        g2 = np.concatenate([wq_s[:, 128:192], wk_s[:, 128:192]], axis=1)
        wqk_h = (np.stack([g0, g1, g2], 0)
                 .reshape(3, NCH, 128, 128).transpose(2, 0, 1, 3)
                 .reshape(128, 3 * NCH * 128))
        wv_h = (Wv[:, sl].reshape(NCH, 128, 192).transpose(1, 0, 2)
                .reshape(128, NCH * 192))
        wo01_h = Wo[sl, :][0:128, :]
        wo2_h = Wo[sl, :][128:192, :]
        bqk_h = np.stack(
            [bq[sl][0:128], bk[sl][0:128],
             np.concatenate([bq[sl][128:192], bk[sl][128:192]])], axis=1
        )  # [128, 3]
        bv_h = bv[sl].reshape(1, 192)
        in_maps.append({
            "xT": xTs[b],
            "wqk": np.ascontiguousarray(wqk_h.astype(mmd_np)),
            "wv": np.ascontiguousarray(wv_h.astype(mmd_np)),
            "wo01": np.ascontiguousarray(wo01_h.astype(mmd_np)),
            "wo2": np.ascontiguousarray(wo2_h.astype(mmd_np)),
            "bqk": np.ascontiguousarray(bqk_h),
            "bv": np.ascontiguousarray(bv_h),
        })

    trace = bool(os.environ.get("KERNEL_TRACE"))
    if trace:
        _install_trace_hooks()
    res = bass_utils.run_bass_kernel_spmd(
        nc, in_maps, core_ids=list(range(NCORES)), trace=trace
    )
    _cache["last_results"] = res

    out = np.empty((B, T, C), dtype=np.float32)
    for b in range(B):
        acc = res.results[b * 4]["y"].copy()
        for hg in range(1, 4):
            acc += res.results[b * 4 + hg]["y"]
        out[b] = acc.T + bo
    return out



# revision 36
# speedup vs baseline: 1.2341x; 1.0132x over previous
"""Causal self-attention (q/k-swapped variant) Bass kernel for Trainium2.

Problem: B=2, T=2048, C=768, H=12, hs=64.
    k = x@Wk+bk ; q = x@Wq+bq ; v = x@Wv+bv          (per-head split)
    att[b,h,i,j] = (k[b,i,h,:] . q[b,j,h,:]) / 8     (note: k rows, q cols)
    att = softmax(causal-mask(att), axis=j)
    y = (att @ v) @ Wo + bo

Sharding: 8 cores = 2 batches x 4 head-groups (3 heads each).
Each core computes its 3 heads fully (QKV proj -> attention -> partial
output projection); host sums the 4 partial outputs per batch and adds bo.

All on-device score math is done in "transposed score" space: score tiles
have j (softmax axis) on partitions and i on the free dim, so the PV matmul
needs no transposes at all, and the softmax denominator falls out of the PV
matmul via an appended ones-column on V.
"""

import os
import sys

sys.path.insert(0, "/opt/trn_rl_repo")

import numpy as np

T = 2048
C = 768
HS = 64
HPC = 3          # heads per core
NCH = C // 128   # 6 contraction chunks
TB = T // 128    # 16 row blocks
JB = T // 128    # 16 j blocks
NCORES = 8
MM_DTYPE = os.environ.get("KERNEL_MM_DTYPE", "fp16")  # fp16 | bf16 | fp32

_cache = {}


def _segments(lo, hi):
    """Split [lo, hi) at 512 boundaries (PSUM bank / fp32 matmul N limit)."""
    out = []
    s = lo
    while s < hi:
        e = min((s // 512 + 1) * 512, hi)
        out.append((s, e))
        s = e
    return out


def _emit(ctx, tc):
    import concourse.bass as bass
    import concourse.tile as tile  # noqa: F401
    from concourse import mybir
    from concourse.bass import ts
    from concourse.masks import make_upper_triangular

    f32 = mybir.dt.float32
    mmd = {"fp16": mybir.dt.float16, "bf16": mybir.dt.bfloat16,
           "fp32": f32}[MM_DTYPE]  # matmul-input dtype
    nc = tc.nc

    xT = nc.dram_tensor("xT", (C, T), mmd, kind="ExternalInput").ap()
    wqk = nc.dram_tensor("wqk", (128, 3 * NCH * 128), mmd, kind="ExternalInput").ap()
    wv = nc.dram_tensor("wv", (128, NCH * 192), mmd, kind="ExternalInput").ap()
    wo01 = nc.dram_tensor("wo01", (128, C), mmd, kind="ExternalInput").ap()
    wo2 = nc.dram_tensor("wo2", (64, C), mmd, kind="ExternalInput").ap()
    bqk = nc.dram_tensor("bqk", (128, 3), f32, kind="ExternalInput").ap()
    bv = nc.dram_tensor("bv", (1, 192), f32, kind="ExternalInput").ap()
    y = nc.dram_tensor("y", (C, T), f32, kind="ExternalOutput").ap()  # transposed

    consts = ctx.enter_context(tc.tile_pool(name="consts", bufs=1))

    # ---- load inputs (wqk + xT first: they gate the first projections) ----
    wqk_sb = consts.tile([128, 3, NCH, 128], mmd)
    nc.sync.dma_start(wqk_sb[:], wqk.rearrange("p (g k m) -> p g k m", g=3, k=NCH))
    xT_sb = consts.tile([128, NCH, T], mmd)
    for k in range(NCH):
        nc.sync.dma_start(xT_sb[:, k, :], xT[k * 128:(k + 1) * 128, :])
    wv_sb = consts.tile([128, NCH, 192], mmd)
    nc.sync.dma_start(wv_sb[:], wv.rearrange("p (k m) -> p k m", k=NCH))
    wo01_sb = consts.tile([128, C], mmd)
    nc.sync.dma_start(wo01_sb[:], wo01)
    wo2_sb = consts.tile([64, C], mmd)
    nc.sync.dma_start(wo2_sb[:], wo2)
    bqk_sb = consts.tile([128, 3], f32)       # per-partition bias per QK group
    nc.sync.dma_start(bqk_sb[:], bqk)
    bvb_sb = consts.tile([128, 192], f32)     # bv broadcast across partitions
    nc.sync.dma_start(bvb_sb[:], bv.to_broadcast((128, 192)))

    scratch = consts.tile([128, 512], mmd)
    nc.vector.memset(scratch[:], 0.0)
    ones64 = consts.tile([1, 64], f32)
    nc.vector.memset(ones64[:], 1.0)
    trimask = consts.tile([128, 128], mmd)
    make_upper_triangular(nc, trimask[:], val=1.0, diag=True)

    V_aug = consts.tile([128, TB, HPC * 65], mmd)
    for h in range(HPC):
        nc.vector.memset(V_aug[:, :, h * 65 + 64:h * 65 + 65], 1.0)

    QK_sb = consts.tile([128, 3, T], mmd)     # g0=Q(h0,h1) g1=K(h0,h1) g2=[Q(h2)|K(h2)]
    KT2_sb = consts.tile([64, T], mmd)        # K(h2) shifted to base partition 0
    AT01_sb = consts.tile([128, T], mmd)      # normalized attn out, h0 rows 0:64, h1 64:128
    AT2_sb = consts.tile([64, T], mmd)        # normalized attn out, h2

    # ---- single fused pipeline ----
    # PSUM: psP (proj/outproj, 2 banks) + psS (scores, 4) + psO (Onum, 2) = 8
    psP = ctx.enter_context(tc.tile_pool(name="psP", bufs=2, space="PSUM"))
    psS = ctx.enter_context(tc.tile_pool(name="psS", bufs=2, space="PSUM"))
    psO = ctx.enter_context(tc.tile_pool(name="psO", bufs=1, space="PSUM"))
    sbE = ctx.enter_context(tc.tile_pool(name="E", bufs=5))
    sbATn = ctx.enter_context(tc.tile_pool(name="ATn", bufs=2))
    sbRZ = ctx.enter_context(tc.tile_pool(name="RZ", bufs=2))
    sbY = ctx.enter_context(tc.tile_pool(name="Y", bufs=4))

    # PE warm-up (keeps HAM at full clock while inputs stream in) + exp
    # table pre-load
    for _ in range(28):
        warm = psP.tile([128, 512], f32, tag="p")
        nc.tensor.matmul(warm[:], lhsT=scratch[:, 0:128], rhs=scratch[:],
                         start=True, stop=True, skip_group_check=True)
    edum = sbE.tile([128, 1024], mmd)
    nc.scalar.activation(edum[:, 0:512], scratch[:],
                         mybir.ActivationFunctionType.Exp, scale=0.125)

    def qk_mm(g, it):
        ps = psP.tile([128, 512], f32, tag="p")
        for k in range(NCH):
            nc.tensor.matmul(ps[:], lhsT=wqk_sb[:, g, k, :],
                             rhs=xT_sb[:, k, ts(it, 512)],
                             start=(k == 0), stop=(k == NCH - 1))
        return ps

    def qk_epi(ps, g, it):
        nc.vector.tensor_add(QK_sb[:, g, ts(it, 512)], ps[:],
                             bqk_sb[:, g:g + 1].to_broadcast((128, 512)))

    def qk_group(g, it):
        qk_epi(qk_mm(g, it), g, it)

    def v_mm(tb):
        ps = psP.tile([128, 512], f32, tag="p")
        for k in range(NCH):
            nc.tensor.matmul(ps[:, 0:192], lhsT=xT_sb[:, k, ts(tb, 128)],
                             rhs=wv_sb[:, k, :],
                             start=(k == 0), stop=(k == NCH - 1))
        return ps

    def v_epi(ps, tb):
        for h in range(HPC):
            nc.vector.tensor_add(V_aug[:, tb, h * 65:h * 65 + 64],
                                 ps[:, h * 64:(h + 1) * 64],
                                 bvb_sb[:, h * 64:(h + 1) * 64])

    def v_group(tb):
        v_epi(v_mm(tb), tb)

    def op_mm(cb, tt):
        ps = psP.tile([128, 512], f32, tag="p")
        nc.tensor.matmul(ps[:], lhsT=wo01_sb[:, ts(cb, 128)],
                         rhs=AT01_sb[:, ts(tt, 512)], start=True, stop=False)
        nc.tensor.matmul(ps[:], lhsT=wo2_sb[:, ts(cb, 128)],
                         rhs=AT2_sb[:, ts(tt, 512)], start=False, stop=True)
        return ps

    def op_epi(ps, cb, tt):
        ysb = sbY.tile([128, 512], f32)
        nc.vector.tensor_copy(ysb[:], ps[:])
        nc.sync.dma_start(y[cb * 128:(cb + 1) * 128, tt * 512:(tt + 1) * 512],
                          ysb[:])

    def kt2_shift(lo, hi):
        nc.sync.dma_start(KT2_sb[:, lo:hi], QK_sb[64:128, 2, lo:hi])

    # pre-phase: the minimum the first unit needs before its first chunk;
    # the rest of the first unit's inputs arrive as inline-epilogue fillers
    # popped from inside its chunk loop (deadlines: qk(0,1) by pop 2, v(tb)
    # by pop tb+1).
    for it in range(2):
        qk_group(1, it)
    qk_group(0, 0)
    v_group(0)

    from collections import deque
    # Filler groups are (mm_emitter, epi_maker) pairs. The mm stage runs PE
    # matmuls; the epilogue (a DVE bias-add / PSUM->SBUF copy) is deferred
    # two pops so it never enqueues on DVE while its matmuls are still in
    # flight (head-of-line blocking on the strict-FIFO DVE queue).
    # Emission-order invariants (Tile deps follow emission order):
    #  - qk(2,*)+kt2_shift before the half0-h2 unit (its STs read g2/KT2);
    #    they sit at pops 10-14, i.e. during half0-h1 -> barrier is a no-op.
    #  - qk(1,2..3) epi before half1's first ST (chunk 24): pops 3/5 +2.
    #  - v(tb) epi before the first PV reading V_aug[:,tb]: v8-15 pop at
    #    chunks 15-22, first consumer is half1-h1's PV(8) around chunk 35.
    def G(mm, epi=None):
        return (mm, epi)

    def qk_G(g, it):
        return G(lambda: qk_mm(g, it), lambda ps: qk_epi(ps, g, it))

    def v_G(tb):
        return G(lambda: v_mm(tb), lambda ps: v_epi(ps, tb))

    # first-unit fillers run with INLINE epilogues (one full group per pop)
    # so the deadline-dense v1..v7 bias-adds land before the PVs that read
    # them; DVE head-of-line cost is negligible while the pipe fills.
    first_fillers = deque(
        [G(lambda: (qk_group(0, 1), None)[1])]
        + [G(lambda tb=tb: (v_group(tb), None)[1]) for tb in range(1, 8)]
    )
    # kt2 shifts sit >=2 items after the qk(2,*) epis they read, so the
    # due-discipline of the epi backlog emits those adds first
    pre_fillers = deque(
        [qk_G(1, 2), qk_G(1, 3), qk_G(2, 0), qk_G(2, 1),
         qk_G(2, 2), qk_G(2, 3),
         G(lambda: (kt2_shift(0, 1024), None)[1]),
         G(lambda: (kt2_shift(1024, 2048), None)[1])]
    )
    qk0_late = deque([qk_G(0, 2), qk_G(0, 3)])
    v_late = deque([v_G(tb) for tb in range(8, TB)])
    op_fillers = deque()
    epi_backlog = deque()   # (due_popcount, thunk)
    popcnt = [0]

    def pop_filler():
        popcnt[0] += 1
        if epi_backlog and epi_backlog[0][0] <= popcnt[0]:
            epi_backlog.popleft()[1]()
            return True
        for q in (first_fillers, pre_fillers, qk0_late, v_late, op_fillers):
            if q:
                mm, epi = q.popleft()
                ps = mm()
                if epi is not None:
                    epi_backlog.append((popcnt[0] + 2, lambda: epi(ps)))
                return True
        if epi_backlog:
            epi_backlog.popleft()[1]()
            return True
        return False

    def flush_fillers(queues):
        for q in queues:
            while q:
                pop_filler()
        while epi_backlog:
            epi_backlog.popleft()[1]()

    # per-head (lhsT=Q^T, rhs=K^T) access patterns; partition bases match
    heads = [
        (QK_sb[0:64, 0, :], QK_sb[0:64, 1, :]),
        (QK_sb[64:128, 0, :], QK_sb[64:128, 1, :]),
        (QK_sb[0:64, 2, :], KT2_sb[:, :]),
    ]

    HW = 1024  # i-window per (half, head) unit
    # half1 runs h1 first so its AT-shift DMA clears long before the tail,
    # and ends on h0 whose normalization chain has no trailing shift.
    head_order = [(0, 1, 2), (1, 2, 0)]
    for half in range(T // HW):
        c0 = HW * half
        njb = (c0 + HW) // 128
        for h in head_order[half]:
            if h == 2 and (pre_fillers or epi_backlog):
                # h2 reads g2/KT2: force their writers (and any pending
                # epilogues) out now; normally a no-op by pop scheduling
                flush_fillers((pre_fillers,))
            QT, KT = heads[h]
            Onum = psO.tile([65, HW], f32)

            def emit_pv(jb, E, lo):
                for a, b in _segments(lo, c0 + HW):
                    nc.tensor.matmul(Onum[:, a - c0:b - c0],
                                     lhsT=V_aug[:, jb, h * 65:(h + 1) * 65],
                                     rhs=E[:, a - c0:b - c0],
                                     start=(jb == 0),
                                     stop=(jb == min(4 * (a // 512) + 3,
                                                     njb - 1)),
                                     skip_group_check=True)

            # the last unit's first 512-col segment finishes accumulating
            # at jb = 4*(c0//512)+3: launch its normalization chain early so
            # its output projection overlaps the rest of the unit
            last_unit = (half == 1 and h == head_order[1][-1])
            segA_last_jb = min(4 * (c0 // 512) + 3, njb - 1)

            def emit_chain_seg(s0, s1):
                """normalize Onum[:, s0:s1] -> AT01[0:64, c0+s0:c0+s1].
                1/Z via exp(-ln Z) on the scalar engine straight from the
                PSUM Z row: the ACT is idle at the tail and this skips both
                DMA-reshape round-trips (~3us queue latency each)."""
                w = s1 - s0
                lnz = sbRZ.tile([1, 512], f32, tag="lnzs")
                nc.scalar.activation(lnz[:, 0:w], Onum[64:65, s0:s1],
                                     mybir.ActivationFunctionType.Ln)
                rz1s = sbRZ.tile([1, 512], f32, tag="rz1s")
                nc.scalar.activation(rz1s[:, 0:w], lnz[:, 0:w],
                                     mybir.ActivationFunctionType.Exp,
                                     scale=-1.0)
                ATs = sbATn.tile([65, 512], f32, tag="a512")
                nc.vector.tensor_copy(ATs[:, 0:w], Onum[:, s0:s1])
                rzbs = sbRZ.tile([64, 512], f32, tag="rzbs")
                nc.gpsimd.partition_broadcast(rzbs[:], rz1s[:, 0:w],
                                              channels=64)
                nc.vector.tensor_mul(AT01_sb[0:64, c0 + s0:c0 + s1],
                                     ATs[0:64, 0:w], rzbs[:])

            pending = []
            for jb in range(njb):
                i0 = 128 * jb
                lo = max(c0, i0)
                S = psS.tile([128, HW], f32)
                for a, b in _segments(lo, c0 + HW):
                    nc.tensor.matmul(S[:, a - c0:b - c0],
                                     lhsT=QT[:, ts(jb, 128)],
                                     rhs=KT[:, a:b], start=True, stop=True)
                E = sbE.tile([128, HW], mmd)
                nc.scalar.activation(E[:, lo - c0:], S[:, lo - c0:],
                                     mybir.ActivationFunctionType.Exp,
                                     scale=0.125)
                if lo == i0:  # window containing the diagonal block
                    r = i0 - c0
                    nc.vector.tensor_mul(E[:, r:r + 128], E[:, r:r + 128],
                                         trimask[:])
                if not pop_filler():
                    # one dummy matmul per filler miss keeps the PE activity
                    # monitor from re-throttling the clock mid-attention
                    warm = psP.tile([128, 512], f32, tag="p")
                    nc.tensor.matmul(warm[:], lhsT=scratch[:, 0:128],
                                     rhs=scratch[:], start=True, stop=True,
                                     skip_group_check=True)
                pending.append((jb, E, lo))
                if len(pending) > 3:  # PV trails ST by 3 chunks
                    done = pending.pop(0)
                    emit_pv(*done)
                    if last_unit and done[0] == segA_last_jb:
                        emit_chain_seg(0, 512)
            for item in pending:
                emit_pv(*item)
                if last_unit and item[0] == segA_last_jb:
                    emit_chain_seg(0, 512)

            if last_unit:
                emit_chain_seg(512, HW)
                continue

            # row 64 of Onum is Z, on one partition. DMA-reshape it straight
            # from PSUM to [128, HW/128] (overlapping the ATn copy) for a
            # parallel DVE reciprocal, fold back, then gpsimd replicates 1/Z
            # across 64 partitions for the divide.
            ATn = sbATn.tile([65, HW], f32)
            nc.vector.tensor_copy(ATn[:], Onum[:])
            z16 = sbRZ.tile([128, HW // 128], f32, tag="z16")
            nc.sync.dma_start(z16[:], ATn[64:65, :])
            r16 = sbRZ.tile([128, HW // 128], f32, tag="r16")
            nc.vector.reciprocal(r16[:], z16[:])
            rz1 = sbRZ.tile([1, HW], f32, tag="rz1")
            nc.sync.dma_start(rz1[:], r16[:])
            rzb = sbRZ.tile([64, HW], f32, tag="rzb")
            nc.gpsimd.partition_broadcast(rzb[:], rz1[:], channels=64)
            if h == 0:
                nc.vector.tensor_mul(AT01_sb[0:64, c0:c0 + HW], ATn[0:64, :],
                                     rzb[:])
            elif h == 2:
                nc.vector.tensor_mul(AT2_sb[:, c0:c0 + HW], ATn[0:64, :],
                                     rzb[:])
            else:
                # h1's rows live at partitions 64:128 of AT01: normalize into
                # a scratch tile, then partition-shift via SBUF-to-SBUF DMA.
                ATsh = sbRZ.tile([64, HW], mmd, tag="atsh")
                nc.vector.tensor_mul(ATsh[:], ATn[0:64, :], rzb[:])
                nc.sync.dma_start(AT01_sb[64:128, c0:c0 + HW], ATsh[:])

        # all heads done for this half: its output columns can project out;
        # groups run as fillers inside the next half (or drain at the end)
        for tt in range(c0 // 512, (c0 + HW) // 512):
            for cb in range(NCH):
                op_fillers.append(
                    G(lambda cb=cb, tt=tt: op_mm(cb, tt),
                      lambda ps, cb=cb, tt=tt: op_epi(ps, cb, tt)))

    # drain remaining fillers (the last half's output projection); a few
    # dummies bridge the last normalization chain so the PE stays warm
    for _ in range(14):
        warm = psP.tile([128, 512], f32, tag="p")
        nc.tensor.matmul(warm[:], lhsT=scratch[:, 0:128], rhs=scratch[:],
                         start=True, stop=True, skip_group_check=True)
    flush_fillers((first_fillers, pre_fillers, qk0_late, v_late,
                   op_fillers))


def _build():
    if "nc" in _cache:
        return _cache["nc"]
    from contextlib import ExitStack

    import concourse.tile as tile
    from concourse import bacc

    nc = bacc.Bacc("TRN2", target_bir_lowering=False, debug=False,
                   num_devices=NCORES)
    with tile.TileContext(nc) as tc:
        with ExitStack() as ctx:
            _emit(ctx, tc)
    nc.compile()
    _cache["nc"] = nc
    return nc


def _install_trace_hooks():
    """Make trace=True work in this container: shim the missing
    antenv.axon_hooks NTFF-profile hook (ctypes into libaxon_pjrt.so) and
    skip the S3 artifact upload."""
    import contextlib
    import ctypes
    import types

    import concourse.bass_utils as bu

    bu.upload_artifacts = lambda tmpdir: tmpdir
    try:
        from antenv.axon_hooks import get_axon_ntff_profile_hook  # noqa: F401
        return
    except ImportError:
        pass

    so_path = "/opt/axon/libaxon_pjrt.so"
    if not os.path.exists(so_path):
        return
    lib = ctypes.CDLL(so_path)
    if not hasattr(lib, "axon_start_nrt_profile"):
        return
    lib.axon_start_nrt_profile.argtypes = [
        ctypes.POINTER(ctypes.c_int64), ctypes.c_size_t,
    ]
    lib.axon_start_nrt_profile.restype = ctypes.c_int64
    lib.axon_stop_nrt_profile.argtypes = [ctypes.c_char_p]
    lib.axon_stop_nrt_profile.restype = ctypes.c_int64

    @contextlib.contextmanager
    def _hook(output_dir, device_ids):
        import jax
        jax.devices()
        if device_ids:
            ids = (ctypes.c_int64 * len(device_ids))(*device_ids)
            rc = lib.axon_start_nrt_profile(ids, len(device_ids))
        else:
            rc = lib.axon_start_nrt_profile(None, 0)
        if rc != 0:
            raise RuntimeError(f"axon_start_nrt_profile rc={rc}")
        try:
            yield
        finally:
            n = lib.axon_stop_nrt_profile(str(output_dir).encode())
            print(f"profile: {n} file(s) written to {output_dir}",
                  file=sys.stderr)

    state = {"h": _hook}
    mod = types.ModuleType("antenv.axon_hooks")
    mod.get_axon_ntff_profile_hook = lambda: state["h"]
    mod.set_axon_ntff_profile_hook = lambda h: state.__setitem__("h", h)
    import antenv
    antenv.axon_hooks = mod
    sys.modules["antenv.axon_hooks"] = mod


def kernel(**inputs):
    x = np.ascontiguousarray(np.asarray(inputs["x"], dtype=np.float32))
    Wq = np.asarray(inputs["Wq"], dtype=np.float32)
    Wk = np.asarray(inputs["Wk"], dtype=np.float32)
    Wv = np.asarray(inputs["Wv"], dtype=np.float32)
    Wo = np.asarray(inputs["Wo"], dtype=np.float32)
    bq = np.asarray(inputs["bq"], dtype=np.float32)
    bk = np.asarray(inputs["bk"], dtype=np.float32)
    bv = np.asarray(inputs["bv"], dtype=np.float32)
    bo = np.asarray(inputs["bo"], dtype=np.float32)

    from concourse import bass_utils

    nc = _build()

    if MM_DTYPE == "bf16":
        import ml_dtypes
        mmd_np = ml_dtypes.bfloat16
    elif MM_DTYPE == "fp16":
        mmd_np = np.float16
    else:
        mmd_np = np.float32

    B = x.shape[0]
    xTs = [np.ascontiguousarray(x[b].T.astype(mmd_np)) for b in range(B)]
    in_maps = []
    for core in range(NCORES):
        b, hg = core // 4, core % 4
        sl = slice(hg * 192, (hg + 1) * 192)
        wq_s, wk_s = Wq[:, sl], Wk[:, sl]
        g0 = wq_s[:, 0:128]
        g1 = wk_s[:, 0:128]
        g2 = np.concatenate([wq_s[:, 128:192], wk_s[:, 128:192]], axis=1)
        wqk_h = (np.stack([g0, g1, g2], 0)
                 .reshape(3, NCH, 128, 128).transpose(2, 0, 1, 3)
                 .reshape(128, 3 * NCH * 128))
        wv_h = (Wv[:, sl].reshape(NCH, 128, 192).transpose(1, 0, 2)
                .reshape(128, NCH * 192))
        wo01_h = Wo[sl, :][0:128, :]
        wo2_h = Wo[sl, :][128:192, :]
        bqk_h = np.stack(
            [bq[sl][0:128], bk[sl][0:128],
             np.concatenate([bq[sl][128:192], bk[sl][128:192]])], axis=1
        )  # [128, 3]
        bv_h = bv[sl].reshape(1, 192)
        in_maps.append({
            "xT": xTs[b],
            "wqk": np.ascontiguousarray(wqk_h.astype(mmd_np)),
            "wv": np.ascontiguousarray(wv_h.astype(mmd_np)),
            "wo01": np.ascontiguousarray(wo01_h.astype(mmd_np)),
            "wo2": np.ascontiguousarray(wo2_h.astype(mmd_np)),
            "bqk": np.ascontiguousarray(bqk_h),
            "bv": np.ascontiguousarray(bv_h),
        })

    trace = bool(os.environ.get("KERNEL_TRACE"))
    if trace:
        _install_trace_hooks()
    res = bass_utils.run_bass_kernel_spmd(
        nc, in_maps, core_ids=list(range(NCORES)), trace=trace
    )
    _cache["last_results"] = res

    out = np.empty((B, T, C), dtype=np.float32)
    for b in range(B):
        acc = res.results[b * 4]["y"].copy()
        for hg in range(1, 4):
            acc += res.results[b * 4 + hg]["y"]
        out[b] = acc.T + bo
    return out

